# revision 14
# baseline (speedup 1.0000x reference)
"""ChannelAttention (LKA3D) Trainium2 Bass kernel, v4.

Problem: B=4, N=16384, C=384, heads=4, head_dim=96.
Reference: qkv = x @ W_qkv.T; per head q,k,v transposed to (d, N);
q,k L2-normalized over N; attn = softmax((q@k.T)*temp, axis=-1);
out = (attn @ v) reassembled to (B,N,C) @ W_out.T + b_out.

Structure: channel attention only needs the d x d gram and per-channel
norms, all bilinear in x. With S = X.T @ X (384x384):
  G_h       = Wq_h.T S Wk_h          (per-head 96x96 logits)
  ||q_c||^2 = diag(Wq.T S Wq),  ||k_d||^2 = diag(Wk.T S Wk)
and the output path folds completely:
  out = V P = X (Wv.T P) = X M,  M = Wv.T (blockdiag(attn).T Wo)

Sharding: core c handles batch c//2, token half c%2 (8192 tokens); a
2-party AllReduce per pair [[0,1],[2,3],[4,5],[6,7]] of the 147KB
stats block (G + norm sums) completes the statistics. The stats chain
runs once per core.

Dtypes (validated vs reference in numpy: rel err 3.7e-3, budget 2e-2):
S from fp8e4 x with DoubleRow matmuls; stats in bf16; out phase bf16 X
/ bf16 M; output downloaded bf16 and upcast on host.

v4 scheduling (from TimelineSim of v3): x loads split in 4 chunks and
weights packed into 2 DMAs so the S build starts ~3us in; activation
function-set loads (1.3us each) pulled off the critical path with dummy
Sqrt/Exp activations; norm rsqrt restructured from [1,768] row ops to
[96,384] wide ops via an outer-product matmul, with temperature folded
into the Exp's per-partition scale; diag products split Pool/DVE;
output stored in 2048-token chunks to shrink the drain tail; loads on
the SP queue, stores on Act/Pool so loop iterations don't block.
"""

import numpy as np
import concourse.bacc as bacc
import concourse.mybir as mybir
from concourse import tile
from concourse.bass_utils import run_bass_kernel_spmd

F32 = mybir.dt.float32
F32R = mybir.dt.float32r
BF16 = mybir.dt.bfloat16
F8 = mybir.dt.float8e4
ALU = mybir.AluOpType
ACTF = mybir.ActivationFunctionType
DR = mybir.MatmulPerfMode.DoubleRow

B = 4
C = 384
NHEADS = 4
DH = 96
NCORES = 8
NFULL = 16384
NL = NFULL // 2        # 8192 tokens per core (one half of one batch)
NK = NL // 256         # 32 fp8 double-row pair tiles
NTC = NL // 512        # 16 out-phase token chunks
NOC = NL // 2048       # 4 output store chunks per ms
STATS_ROWS = 98        # 96 G rows + 2 sq rows


def build_nc(loop_n=1, use_collective=True):
    nc = bacc.Bacc(None, target_bir_lowering=False, debug=False)
    XT8 = nc.dram_tensor("xt8", [NK, 128, 2, C], F8, kind="ExternalInput")
    XCM = nc.dram_tensor("xcm", [3, 128, NL], BF16, kind="ExternalInput")
    WQK = nc.dram_tensor("wqk", [2, 3, 128, C], F32R, kind="ExternalInput")
    WVO = nc.dram_tensor("wvo", [96, 8, C], F32R, kind="ExternalInput")
    BIAS = nc.dram_tensor("bias", [3, 128], F32, kind="ExternalInput")
    TEMP = nc.dram_tensor("temp", [NHEADS], F32R, kind="ExternalInput")
    OUT = nc.dram_tensor("out", [3, 128, NL], BF16, kind="ExternalOutput")
    STATS_IN = nc.dram_tensor("stats_in", [STATS_ROWS * C], F32)
    STATS_OUT = nc.dram_tensor("stats_out", [STATS_ROWS * C], F32)

    def stats_view(t):
        return t.ap().rearrange("(p f) -> p f", p=STATS_ROWS)

    with tile.TileContext(nc) as tc:
        with (
            tc.tile_pool(name="wpool", bufs=1) as wpool,
            tc.tile_pool(name="xpool", bufs=1) as xpool,
            tc.tile_pool(name="spool", bufs=2) as spool,
            tc.tile_pool(name="apool", bufs=2) as apool,
            tc.tile_pool(name="gpool", bufs=2) as gpool,
            tc.tile_pool(name="p2pool", bufs=2) as p2pool,
            tc.tile_pool(name="opool", bufs=2) as opool,
            tc.tile_pool(name="pS", bufs=1, space="PSUM") as pS,
            tc.tile_pool(name="pA", bufs=2, space="PSUM") as pA,
            tc.tile_pool(name="pO", bufs=3, space="PSUM") as pO,
        ):
            # ---- weights: loaded once (resident across loop iterations) ----
            wqk_sb = wpool.tile([128, 2, 3, C], F32R, name="wqk", tag="wqk")
            nc.scalar.dma_start(
                out=wqk_sb[:, :, :, :],
                in_=WQK.ap().rearrange("a b p c -> p a b c"),
            )
            wq_sb, wk_sb = wqk_sb[:, 0], wqk_sb[:, 1]
            wvo_sb = wpool.tile([96, 8, C], F32R, name="wvo", tag="wvo")
            nc.scalar.dma_start(out=wvo_sb[:, :, :], in_=WVO.ap())
            bias_ch = wpool.tile([128, 3], F32, name="bias", tag="bias")
            nc.scalar.dma_start(out=bias_ch[:, :], in_=BIAS.ap().rearrange("a p -> p a"))
            temp_sb = wpool.tile([1, NHEADS], F32R, name="temp", tag="temp")
            nc.scalar.dma_start(out=temp_sb[:, :], in_=TEMP.ap()[None, :])
            ones = wpool.tile([128, 1], BF16, name="ones", tag="ones")
            nc.vector.memset(ones[:, :], 1.0)
            ones_row = wpool.tile([1, 96], F32R, name="onesr", tag="onesr")
            nc.vector.memset(ones_row[:, :].bitcast(F32), 1.0)
            wq_bf = wpool.tile([128, 3, C], BF16, name="wqbf", tag="wqbf")
            wk_bf = wpool.tile([128, 3, C], BF16, name="wkbf", tag="wkbf")
            nc.vector.tensor_copy(wq_bf[:, :, :], wq_sb[:, :, :].bitcast(F32))
            nc.vector.tensor_copy(wk_bf[:, :, :], wk_sb[:, :, :].bitcast(F32))
            # temp broadcast to 96 partitions: temp_col = ones_row.T @ temp
            ptc = pA.tile([96, NHEADS], F32, name="ptc", tag="pa")
            nc.tensor.matmul(
                ptc[:, :], ones_row[:, :], temp_sb[:, :],
                start=True, stop=True,
            )
            temp_col = wpool.tile([96, NHEADS], F32, name="tcol", tag="tcol")
            nc.scalar.copy(temp_col[:, :], ptc[:, :])
            # act-table priming: start in the sqrt set
            dummy = wpool.tile([1, 8], F32, name="dummy", tag="dummy")
            nc.vector.memset(dummy[:, :], 1.0)
            dscr = wpool.tile([1, 8], F32, name="dscr", tag="dscr")
            nc.scalar.sqrt(dscr[:, :], dummy[:, :])

            def body():
                # ---- input loads (inside body for loop-timing parity) ------
                xt8 = xpool.tile([128, NK, 2, C], F8, name="xt8", tag="xt8")
                for i in range(4):
                    nc.sync.dma_start(
                        out=xt8[:, i * 8 : (i + 1) * 8, :, :],
                        in_=XT8.ap()[i * 8 : (i + 1) * 8].rearrange(
                            "k p t c -> p k t c"
                        ),
                    )
                xcm = xpool.tile([128, 3, NL], BF16, name="xcm", tag="xcm")
                for cc in range(3):
                    nc.sync.dma_start(out=xcm[:, cc, :], in_=XCM.ap()[cc])

                # ---- S = X^T X via fp8 DoubleRow (256 tokens/matmul) -------
                sacc = [
                    pS.tile([128, C], F32, name=f"s{i}", tag=f"ps{i}")
                    for i in range(3)
                ]
                for k in range(NK):
                    for cc in range(3):
                        nc.tensor.matmul(
                            sacc[cc][:, :],
                            xt8[:, k, :, cc * 128 : (cc + 1) * 128],
                            xt8[:, k, :, :],
                            start=(k == 0),
                            stop=(k == NK - 1),
                            perf_mode=DR,
                        )
                s_sb = spool.tile([128, 3, C], F32R, name="s", tag="s")
                nc.scalar.copy(s_sb[:, 0, :], sacc[0][:, :])
                with nc.allow_low_precision(reason="f32r matmul operand"):
                    nc.vector.tensor_copy(s_sb[:, 1, :], sacc[1][:, :])
                nc.scalar.copy(s_sb[:, 2, :], sacc[2][:, :])

                # ---- A = S @ Wk, A' = S @ Wq (bf16 results) ----------------
                ab = {}
                for w_sb, nm in ((wk_sb, "a"), (wq_sb, "ap")):
                    a_bf = apool.tile([128, 3, C], BF16, name=nm, tag=nm)
                    for co in range(3):
                        pa = pA.tile([128, C], F32, name="pa", tag="pa")
                        for e in range(3):
                            nc.tensor.matmul(
                                pa[:, :],
                                s_sb[:, e, co * 128 : (co + 1) * 128],
                                w_sb[:, e, :],
                                start=(e == 0),
                                stop=(e == 2),
                            )
                        if co != 1:
                            nc.scalar.copy(a_bf[:, co, :], pa[:, :])
                        else:
                            nc.vector.tensor_copy(a_bf[:, co, :], pa[:, :])
                    ab[nm] = a_bf
                a_bf, ap_bf = ab["a"], ab["ap"]

                # ---- stats block: G rows 0:96, sq rows 96:98 ---------------
                gacc = gpool.tile([96, C], F32, name="gacc", tag="gacc")
                sqrow = gpool.tile([1, 768], F32, name="sqrow", tag="sqrow")
                for h in range(NHEADS):
                    hs = slice(h * DH, (h + 1) * DH)
                    pg = pA.tile([96, 96], F32, name="pg", tag="pa")
                    for cc in range(3):
                        nc.tensor.matmul(
                            pg[:, :],
                            wq_bf[:, cc, hs],
                            a_bf[:, cc, hs],
                            start=(cc == 0),
                            stop=(cc == 2),
                        )
                    nc.vector.tensor_copy(gacc[:, hs], pg[:, :])
                # diag(W.T S W) via elementwise W*A + ones-matmul
                for j, (w_bf, asb) in enumerate(
                    ((wq_bf, ap_bf), (wk_bf, a_bf))
                ):
                    pd = pA.tile([1, C], F32, name="pd", tag="pa")
                    for cc in range(3):
                        prod = gpool.tile(
                            [128, C], BF16, name=f"prod{j}{cc}", tag=f"prod{j}{cc}"
                        )
                        eng = nc.gpsimd if j == 0 else nc.vector
                        eng.tensor_tensor(
                            prod[:, :], w_bf[:, cc, :], asb[:, cc, :], ALU.mult
                        )
                        nc.tensor.matmul(
                            pd[0:1, :],
                            ones[:, :],
                            prod[:, :],
                            start=(cc == 0),
                            stop=(cc == 2),
                        )
                    nc.vector.tensor_copy(sqrow[0:1, j * C : (j + 1) * C], pd[0:1, :])

                nc.scalar.dma_start(
                    out=stats_view(STATS_IN)[0:96], in_=gacc[:, :]
                )
                nc.scalar.dma_start(
                    out=STATS_IN.ap()[96 * C : STATS_ROWS * C][None, :],
                    in_=sqrow[0:1, :],
                )
                if use_collective:
                    nc.gpsimd.collective_compute(
                        "AllReduce",
                        ALU.add,
                        replica_groups=[[2 * i, 2 * i + 1] for i in range(B)],
                        ins=[STATS_IN.ap().opt()],
                        outs=[STATS_OUT.ap().opt()],
                    )
                stats_src = STATS_OUT if use_collective else STATS_IN
                g_all = gpool.tile([96, C], F32, name="gall", tag="gall")
                sq_all = gpool.tile([1, 768], F32, name="sqall", tag="sqall")
                nc.scalar.dma_start(out=g_all[:, :], in_=stats_view(stats_src)[0:96])
                nc.gpsimd.dma_start(
                    out=sq_all[0:1, :],
                    in_=stats_src.ap()[96 * C : STATS_ROWS * C][None, :],
                )

                # ---- attention weights: logits = G * rsqrt(sqq x sqk) ------
                # psc_sq[c, h*96+d] = sqq[h*96+c] * sqk[h*96+d] via outer mm
                sq_bf = gpool.tile([1, 768], BF16, name="sqbf", tag="sqbf")
                nc.vector.tensor_copy(sq_bf[0:1, :], sq_all[0:1, :])
                psq = pA.tile([96, C], F32, name="psq", tag="pa")
                for h in range(NHEADS):
                    hs = slice(h * DH, (h + 1) * DH)
                    nc.tensor.matmul(
                        psq[:, hs],
                        sq_bf[0:1, hs],
                        sq_bf[0:1, 384 + h * DH : 384 + (h + 1) * DH],
                        start=True,
                        stop=True,
                    )
                vs = p2pool.tile([96, C], F32, name="vs", tag="vs")
                nc.vector.tensor_scalar_max(vs[:, :], psq[:, :], 1e-24)
                nc.scalar.sqrt(vs[:, :], vs[:, :])
                # preload the Exp table while DVE computes recip/logits
                nc.scalar.activation(dscr[:, :], dummy[:, :], ACTF.Exp)
                rsqm = p2pool.tile([96, C], F32, name="rsqm", tag="rsqm")
                nc.vector.reciprocal(rsqm[:, :], vs[:, :])
                logit = p2pool.tile([96, C], F32, name="logit", tag="logit")
                nc.vector.tensor_tensor(
                    logit[:, :], g_all[:, :], rsqm[:, :], ALU.mult
                )
                expt = p2pool.tile([96, C], F32, name="expt", tag="expt")
                den = p2pool.tile([96, NHEADS], F32, name="den", tag="den")
                for h in range(NHEADS):
                    hs = slice(h * DH, (h + 1) * DH)
                    nc.scalar.activation(
                        expt[:, hs], logit[:, hs], ACTF.Exp,
                        scale=temp_col[:, h : h + 1],
                        accum_out=den[:, h : h + 1],
                    )
                denr = p2pool.tile([96, NHEADS], F32, name="denr", tag="denr")
                nc.vector.reciprocal(denr[:, :], den[:, :])
                p_sb = p2pool.tile([96, NHEADS, C], F32R, name="p", tag="p")
                for h in range(NHEADS):
                    hs = slice(h * DH, (h + 1) * DH)
                    attn = p2pool.tile([96, 96], F32R, name="attn", tag="attn")
                    with nc.allow_low_precision(reason="f32r matmul operand"):
                        nc.vector.tensor_scalar_mul(
                            attn[:, :], expt[:, hs], denr[:, h : h + 1]
                        )
                    pp = pA.tile([96, C], F32, name="pp", tag="pa")
                    nc.tensor.matmul(
                        pp[:, :], attn[:, :], wvo_sb[:, 4 + h, :],
                        start=True, stop=True,
                    )
                    nc.scalar.copy(p_sb[:, h, :], pp[:, :])
                m_sb = p2pool.tile([128, 3, C], BF16, name="m", tag="m")
                for co in range(3):
                    pm = pA.tile([128, C], F32, name="pm", tag="pa")
                    for h in range(NHEADS):
                        nc.tensor.matmul(
                            pm[:, :],
                            wvo_sb[:, h, co * 128 : (co + 1) * 128],
                            p_sb[:, h, :],
                            start=(h == 0),
                            stop=(h == NHEADS - 1),
                        )
                    nc.scalar.copy(m_sb[:, co, :], pm[:, :])
                # flip the act table back to the sqrt set during the out phase
                nc.scalar.sqrt(dscr[:, :], dummy[:, :])

                # ---- out.T = M.T X (+bias), bf16, 2048-token store chunks --
                for ms in range(3):
                    obuf = opool.tile([128, NL], BF16, name="osb", tag="osb")
                    for tcn in range(NTC):
                        po = pO.tile([128, 512], F32, name="po", tag="po")
                        for cc in range(3):
                            nc.tensor.matmul(
                                po[:, :],
                                m_sb[:, cc, ms * 128 : (ms + 1) * 128],
                                xcm[:, cc, tcn * 512 : (tcn + 1) * 512],
                                start=(cc == 0),
                                stop=(cc == 2),
                            )
                        nc.vector.tensor_scalar_add(
                            obuf[:, tcn * 512 : (tcn + 1) * 512],
                            po[:, :],
                            bias_ch[:, ms : ms + 1],
                        )
                        if tcn % 4 == 3:
                            oc = tcn // 4
                            eng = nc.scalar if (ms * NOC + oc) % 2 == 0 else nc.gpsimd
                            eng.dma_start(
                                out=OUT.ap()[ms, :, oc * 2048 : (oc + 1) * 2048],
                                in_=obuf[:, oc * 2048 : (oc + 1) * 2048],
                            )

                if not use_collective:
                    # traffic-parity stand-in for the AllReduce leg
                    nc.sync.dma_start(out=STATS_OUT.ap(), in_=STATS_IN.ap())

            if loop_n > 1:
                _eng = mybir.EngineType
                with tc.For_i(
                    0, loop_n, 1, staggered_reset=True,
                    hint_engines=(_eng.PE, _eng.DVE, _eng.Activation, _eng.SP,
                                  _eng.Pool),
                ):
                    body()
            else:
                body()

    nc.compile()
    return nc


_NC_CACHE = {}


def get_nc(loop_n=1, use_collective=True):
    key = (loop_n, use_collective)
    if key not in _NC_CACHE:
        _NC_CACHE[key] = build_nc(loop_n, use_collective)
    return _NC_CACHE[key]


def prep_in_maps(x, W_qkv, temperature_ch, W_out, b_out):
    import ml_dtypes

    x = np.asarray(x, np.float32)
    W_qkv = np.asarray(W_qkv, np.float32)
    W_out = np.asarray(W_out, np.float32)
    b_out = np.asarray(b_out, np.float32)
    temp = np.asarray(temperature_ch, np.float32).reshape(-1)
    x8 = x.astype(ml_dtypes.float8_e4m3)
    xcm_full = np.ascontiguousarray(
        x.transpose(0, 2, 1).astype(ml_dtypes.bfloat16)
    )  # (B, C, N)
    wq = np.ascontiguousarray(W_qkv[0:C].T).reshape(3, 128, C)
    wk = np.ascontiguousarray(W_qkv[C : 2 * C].T).reshape(3, 128, C)
    wqk = np.ascontiguousarray(np.stack([wq, wk]))  # (2, 3, 128, C)
    wv = W_qkv[2 * C : 3 * C].reshape(4, 96, C).transpose(1, 0, 2)  # (96,4,C)
    wo = np.stack(
        [W_out[:, h * DH : (h + 1) * DH].T for h in range(4)]
    ).transpose(1, 0, 2)  # (96, 4, C)
    wvo = np.ascontiguousarray(np.concatenate([wv, wo], axis=1))  # (96,8,C)
    shared = {
        "wqk": wqk, "wvo": wvo,
        "bias": np.ascontiguousarray(b_out.reshape(3, 128)), "temp": temp,
    }
    maps = []
    for core in range(NCORES):
        b, half = core // 2, core % 2
        xs = x8[b, half * NL : (half + 1) * NL]  # (NL, C)
        xt8 = np.ascontiguousarray(
            xs.reshape(NK, 2, 128, C).transpose(0, 2, 1, 3)
        )
        xcm = np.ascontiguousarray(
            xcm_full[b, :, half * NL : (half + 1) * NL]
        ).reshape(3, 128, NL)
        maps.append(dict(shared, xt8=xt8, xcm=xcm))
    return maps


def kernel(**inputs):
    nc = get_nc(1)
    in_maps = prep_in_maps(
        inputs["x"],
        inputs["W_qkv"],
        inputs["temperature_ch"],
        inputs["W_out"],
        inputs["b_out"],
    )
    res = run_bass_kernel_spmd(nc, in_maps, core_ids=list(range(NCORES)))
    out = np.empty((B, NFULL, C), np.float32)
    for core in range(NCORES):
        b, half = core // 2, core % 2
        r = res.results[core]["out"]  # (3, 128, NL) bf16, channel-major
        out[b, half * NL : (half + 1) * NL, :] = (
            r.reshape(C, NL).T.astype(np.float32)
        )
    return out


# revision 18
# speedup vs baseline: 2.6812x; 2.6812x over previous
"""ChannelAttention (LKA3D) Trainium2 Bass kernel, v4.

Problem: B=4, N=16384, C=384, heads=4, head_dim=96.
Reference: qkv = x @ W_qkv.T; per head q,k,v transposed to (d, N);
q,k L2-normalized over N; attn = softmax((q@k.T)*temp, axis=-1);
out = (attn @ v) reassembled to (B,N,C) @ W_out.T + b_out.

Structure: channel attention only needs the d x d gram and per-channel
norms, all bilinear in x. With S = X.T @ X (384x384):
  G_h       = Wq_h.T S Wk_h          (per-head 96x96 logits)
  ||q_c||^2 = diag(Wq.T S Wq),  ||k_d||^2 = diag(Wk.T S Wk)
and the output path folds completely:
  out = V P = X (Wv.T P) = X M,  M = Wv.T (blockdiag(attn).T Wo)

Sharding: core c handles batch c//2, token half c%2 (8192 tokens); a
2-party AllReduce per pair [[0,1],[2,3],[4,5],[6,7]] of the 147KB
stats block (G + norm sums) completes the statistics. The stats chain
runs once per core.

Dtypes (validated vs reference in numpy: rel err 3.7e-3, budget 2e-2):
S from fp8e4 x with DoubleRow matmuls; stats in bf16; out phase bf16 X
/ bf16 M; output downloaded bf16 and upcast on host.

v4 scheduling (from TimelineSim of v3): x loads split in 4 chunks and
weights packed into 2 DMAs so the S build starts ~3us in; activation
function-set loads (1.3us each) pulled off the critical path with dummy
Sqrt/Exp activations; norm rsqrt restructured from [1,768] row ops to
[96,384] wide ops via an outer-product matmul, with temperature folded
into the Exp's per-partition scale; diag products split Pool/DVE;
output stored in 2048-token chunks to shrink the drain tail; loads on
the SP queue, stores on Act/Pool so loop iterations don't block.
"""

import numpy as np
import concourse.bacc as bacc
import concourse.mybir as mybir
from concourse import tile
from concourse.bass_utils import run_bass_kernel_spmd

F32 = mybir.dt.float32
F32R = mybir.dt.float32r
BF16 = mybir.dt.bfloat16
F8 = mybir.dt.float8e4
ALU = mybir.AluOpType
ACTF = mybir.ActivationFunctionType
DR = mybir.MatmulPerfMode.DoubleRow

B = 4
C = 384
NHEADS = 4
DH = 96
NCORES = 8
NFULL = 16384
NL = NFULL // 2        # 8192 tokens per core (one half of one batch)
NK = NL // 256         # 32 fp8 double-row pair tiles
NTC = NL // 512        # 16 out-phase token chunks
NOC = NL // 2048       # 4 output store chunks per ms
STATS_ROWS = 98        # 96 G rows + 2 sq rows


def build_nc(loop_n=1, use_collective=True):
    nc = bacc.Bacc(None, target_bir_lowering=False, debug=False)
    XT8 = nc.dram_tensor("xt8", [NK, 128, 2, C], F8, kind="ExternalInput")
    XCM = nc.dram_tensor("xcm", [3, 128, NL], BF16, kind="ExternalInput")
    WQK = nc.dram_tensor("wqk", [2, 3, 128, C], F32R, kind="ExternalInput")
    WVO = nc.dram_tensor("wvo", [96, 8, C], F32R, kind="ExternalInput")
    BIAS = nc.dram_tensor("bias", [3, 128], F32, kind="ExternalInput")
    TEMP = nc.dram_tensor("temp", [NHEADS], F32R, kind="ExternalInput")
    OUT = nc.dram_tensor("out", [3, 128, NL], BF16, kind="ExternalOutput")
    G_IN = nc.dram_tensor("g_in", [96 * C], F32)
    G_OUT = nc.dram_tensor("g_out", [96 * C], F32)
    SQ_IN = nc.dram_tensor("sq_in", [768], F32)
    SQ_OUT = nc.dram_tensor("sq_out", [768], F32)

    def g_view(t):
        return t.ap().rearrange("(p f) -> p f", p=96)

    with tile.TileContext(nc) as tc:
        with (
            tc.tile_pool(name="wpool", bufs=1) as wpool,
            tc.tile_pool(name="xpool", bufs=1) as xpool,
            tc.tile_pool(name="spool", bufs=2) as spool,
            tc.tile_pool(name="apool", bufs=2) as apool,
            tc.tile_pool(name="gpool", bufs=2) as gpool,
            tc.tile_pool(name="p2pool", bufs=2) as p2pool,
            tc.tile_pool(name="opool", bufs=2) as opool,
            tc.tile_pool(name="pS", bufs=1, space="PSUM") as pS,
            tc.tile_pool(name="pA", bufs=2, space="PSUM") as pA,
            tc.tile_pool(name="pO", bufs=3, space="PSUM") as pO,
        ):
            # ---- weights: loaded once (resident across loop iterations) ----
            wqk_sb = wpool.tile([128, 2, 3, C], F32R, name="wqk", tag="wqk")
            nc.scalar.dma_start(
                out=wqk_sb[:, :, :, :],
                in_=WQK.ap().rearrange("a b p c -> p a b c"),
            )
            wq_sb, wk_sb = wqk_sb[:, 0], wqk_sb[:, 1]
            wvo_sb = wpool.tile([96, 8, C], F32R, name="wvo", tag="wvo")
            nc.scalar.dma_start(out=wvo_sb[:, :, :], in_=WVO.ap())
            bias_ch = wpool.tile([128, 3], F32, name="bias", tag="bias")
            nc.scalar.dma_start(out=bias_ch[:, :], in_=BIAS.ap().rearrange("a p -> p a"))
            temp_sb = wpool.tile([1, NHEADS], F32R, name="temp", tag="temp")
            nc.scalar.dma_start(out=temp_sb[:, :], in_=TEMP.ap()[None, :])
            ones = wpool.tile([128, 1], BF16, name="ones", tag="ones")
            nc.vector.memset(ones[:, :], 1.0)
            ones_row = wpool.tile([1, 96], F32R, name="onesr", tag="onesr")
            nc.vector.memset(ones_row[:, :].bitcast(F32), 1.0)
            wq_bf = wpool.tile([128, 3, C], BF16, name="wqbf", tag="wqbf")
            wk_bf = wpool.tile([128, 3, C], BF16, name="wkbf", tag="wkbf")
            nc.vector.tensor_copy(wq_bf[:, :, :], wq_sb[:, :, :].bitcast(F32))
            nc.vector.tensor_copy(wk_bf[:, :, :], wk_sb[:, :, :].bitcast(F32))
            # temp broadcast to 96 partitions: temp_col = ones_row.T @ temp
            ptc = pA.tile([96, NHEADS], F32, name="ptc", tag="pa")
            nc.tensor.matmul(
                ptc[:, :], ones_row[:, :], temp_sb[:, :],
                start=True, stop=True,
            )
            temp_col = wpool.tile([96, NHEADS], F32, name="tcol", tag="tcol")
            nc.scalar.copy(temp_col[:, :], ptc[:, :])
            # act-table priming: start in the sqrt set
            dummy = wpool.tile([1, 8], F32, name="dummy", tag="dummy")
            nc.vector.memset(dummy[:, :], 1.0)
            dscr = wpool.tile([1, 8], F32, name="dscr", tag="dscr")
            nc.scalar.sqrt(dscr[:, :], dummy[:, :])

            def body():
                # ---- input loads (inside body for loop-timing parity) ------
                xt8 = xpool.tile([128, NK, 2, C], F8, name="xt8", tag="xt8")
                for i in range(4):
                    nc.sync.dma_start(
                        out=xt8[:, i * 8 : (i + 1) * 8, :, :],
                        in_=XT8.ap()[i * 8 : (i + 1) * 8].rearrange(
                            "k p t c -> p k t c"
                        ),
                    )
                # 2048-token chunks: a small stats DMA never queues behind
                # more than 1.5us of head-of-line xcm traffic on the bus
                xcm = xpool.tile([128, 3, NL], BF16, name="xcm", tag="xcm")
                for cc in range(3):
                    for i in range(4):
                        sl = slice(i * 2048, (i + 1) * 2048)
                        nc.sync.dma_start(out=xcm[:, cc, sl], in_=XCM.ap()[cc, :, sl])

                # ---- S = X^T X via fp8 DoubleRow (256 tokens/matmul) -------
                sacc = [
                    pS.tile([128, C], F32, name=f"s{i}", tag=f"ps{i}")
                    for i in range(3)
                ]
                for k in range(NK):
                    for cc in range(3):
                        nc.tensor.matmul(
                            sacc[cc][:, :],
                            xt8[:, k, :, cc * 128 : (cc + 1) * 128],
                            xt8[:, k, :, :],
                            start=(k == 0),
                            stop=(k == NK - 1),
                            perf_mode=DR,
                        )
                s_sb = spool.tile([128, 3, C], F32R, name="s", tag="s")
                nc.scalar.copy(s_sb[:, 0, :], sacc[0][:, :])
                with nc.allow_low_precision(reason="f32r matmul operand"):
                    nc.vector.tensor_copy(s_sb[:, 1, :], sacc[1][:, :])
                nc.scalar.copy(s_sb[:, 2, :], sacc[2][:, :])

                # ---- A = S @ Wk, A' = S @ Wq (bf16 results) ----------------
                ab = {}
                for w_sb, nm in ((wk_sb, "a"), (wq_sb, "ap")):
                    a_bf = apool.tile([128, 3, C], BF16, name=nm, tag=nm)
                    for co in range(3):
                        pa = pA.tile([128, C], F32, name="pa", tag="pa")
                        for e in range(3):
                            nc.tensor.matmul(
                                pa[:, :],
                                s_sb[:, e, co * 128 : (co + 1) * 128],
                                w_sb[:, e, :],
                                start=(e == 0),
                                stop=(e == 2),
                            )
                        if co != 1:
                            nc.scalar.copy(a_bf[:, co, :], pa[:, :])
                        else:
                            nc.vector.tensor_copy(a_bf[:, co, :], pa[:, :])
                    ab[nm] = a_bf
                a_bf, ap_bf = ab["a"], ab["ap"]

                # ---- stats block: G rows 0:96, sq rows 96:98 ---------------
                gacc = gpool.tile([96, C], F32, name="gacc", tag="gacc")
                sqrow = gpool.tile([1, 768], F32, name="sqrow", tag="sqrow")
                for h in range(NHEADS):
                    hs = slice(h * DH, (h + 1) * DH)
                    pg = pA.tile([96, 96], F32, name="pg", tag="pa")
                    for cc in range(3):
                        nc.tensor.matmul(
                            pg[:, :],
                            wq_bf[:, cc, hs],
                            a_bf[:, cc, hs],
                            start=(cc == 0),
                            stop=(cc == 2),
                        )
                    nc.vector.tensor_copy(gacc[:, hs], pg[:, :])
                # diag(W.T S W) via elementwise W*A + ones-matmul
                for j, (w_bf, asb) in enumerate(
                    ((wq_bf, ap_bf), (wk_bf, a_bf))
                ):
                    pd = pA.tile([1, C], F32, name="pd", tag="pa")
                    for cc in range(3):
                        prod = gpool.tile(
                            [128, C], BF16, name=f"prod{j}{cc}", tag=f"prod{j}{cc}"
                        )
                        eng = nc.gpsimd if j == 0 else nc.vector
                        eng.tensor_tensor(
                            prod[:, :], w_bf[:, cc, :], asb[:, cc, :], ALU.mult
                        )
                        nc.tensor.matmul(
                            pd[0:1, :],
                            ones[:, :],
                            prod[:, :],
                            start=(cc == 0),
                            stop=(cc == 2),
                        )
                    nc.vector.tensor_copy(sqrow[0:1, j * C : (j + 1) * C], pd[0:1, :])

                # sq reduced first (tiny): its rsqrt chain + the Exp act-table
                # load hide under the big G reduction that follows
                groups = [[2 * i, 2 * i + 1] for i in range(B)]
                nc.scalar.dma_start(out=SQ_IN.ap()[None, :], in_=sqrow[0:1, :])
                if use_collective:
                    nc.gpsimd.collective_compute(
                        "AllReduce", ALU.add, replica_groups=groups,
                        ins=[SQ_IN.ap().opt()], outs=[SQ_OUT.ap().opt()],
                    )
                nc.scalar.dma_start(out=g_view(G_IN), in_=gacc[:, :])
                if use_collective:
                    nc.gpsimd.collective_compute(
                        "AllReduce", ALU.add, replica_groups=groups,
                        ins=[G_IN.ap().opt()], outs=[G_OUT.ap().opt()],
                    )
                g_src = G_OUT if use_collective else G_IN
                sq_src = SQ_OUT if use_collective else SQ_IN
                g_all = gpool.tile([96, C], F32, name="gall", tag="gall")
                sq_all = gpool.tile([1, 768], F32, name="sqall", tag="sqall")
                nc.gpsimd.dma_start(out=sq_all[0:1, :], in_=sq_src.ap()[None, :])
                nc.scalar.dma_start(out=g_all[:, :], in_=g_view(g_src))

                # ---- attention weights: logits = G * rsqrt(sqq x sqk) ------
                # psc_sq[c, h*96+d] = sqq[h*96+c] * sqk[h*96+d] via outer mm
                sq_bf = gpool.tile([1, 768], BF16, name="sqbf", tag="sqbf")
                nc.vector.tensor_copy(sq_bf[0:1, :], sq_all[0:1, :])
                psq = pA.tile([96, C], F32, name="psq", tag="pa")
                for h in range(NHEADS):
                    hs = slice(h * DH, (h + 1) * DH)
                    nc.tensor.matmul(
                        psq[:, hs],
                        sq_bf[0:1, hs],
                        sq_bf[0:1, 384 + h * DH : 384 + (h + 1) * DH],
                        start=True,
                        stop=True,
                    )
                vs = p2pool.tile([96, C], F32, name="vs", tag="vs")
                nc.vector.tensor_scalar_max(vs[:, :], psq[:, :], 1e-24)
                nc.scalar.sqrt(vs[:, :], vs[:, :])
                # preload the Exp table while DVE computes recip/logits
                nc.scalar.activation(dscr[:, :], dummy[:, :], ACTF.Exp)
                rsqm = p2pool.tile([96, C], F32, name="rsqm", tag="rsqm")
                nc.vector.reciprocal(rsqm[:, :], vs[:, :])
                logit = p2pool.tile([96, C], F32, name="logit", tag="logit")
                nc.vector.tensor_tensor(
                    logit[:, :], g_all[:, :], rsqm[:, :], ALU.mult
                )
                expt = p2pool.tile([96, C], F32, name="expt", tag="expt")
                den = p2pool.tile([96, NHEADS], F32, name="den", tag="den")
                for h in range(NHEADS):
                    hs = slice(h * DH, (h + 1) * DH)
                    nc.scalar.activation(
                        expt[:, hs], logit[:, hs], ACTF.Exp,
                        scale=temp_col[:, h : h + 1],
                        accum_out=den[:, h : h + 1],
                    )
                denr = p2pool.tile([96, NHEADS], F32, name="denr", tag="denr")
                nc.vector.reciprocal(denr[:, :], den[:, :])
                p_sb = p2pool.tile([96, NHEADS, C], F32R, name="p", tag="p")
                for h in range(NHEADS):
                    hs = slice(h * DH, (h + 1) * DH)
                    attn = p2pool.tile([96, 96], F32R, name="attn", tag="attn")
                    with nc.allow_low_precision(reason="f32r matmul operand"):
                        nc.vector.tensor_scalar_mul(
                            attn[:, :], expt[:, hs], denr[:, h : h + 1]
                        )
                    pp = pA.tile([96, C], F32, name="pp", tag="pa")
                    nc.tensor.matmul(
                        pp[:, :], attn[:, :], wvo_sb[:, 4 + h, :],
                        start=True, stop=True,
                    )
                    nc.scalar.copy(p_sb[:, h, :], pp[:, :])
                m_sb = p2pool.tile([128, 3, C], BF16, name="m", tag="m")
                for co in range(3):
                    pm = pA.tile([128, C], F32, name="pm", tag="pa")
                    for h in range(NHEADS):
                        nc.tensor.matmul(
                            pm[:, :],
                            wvo_sb[:, h, co * 128 : (co + 1) * 128],
                            p_sb[:, h, :],
                            start=(h == 0),
                            stop=(h == NHEADS - 1),
                        )
                    nc.scalar.copy(m_sb[:, co, :], pm[:, :])
                # flip the act table back to the sqrt set during the out phase
                nc.scalar.sqrt(dscr[:, :], dummy[:, :])

                # ---- out.T = M.T X (+bias), bf16, 2048-token store chunks --
                for ms in range(3):
                    obuf = opool.tile([128, NL], BF16, name="osb", tag="osb")
                    for tcn in range(NTC):
                        po = pO.tile([128, 512], F32, name="po", tag="po")
                        for cc in range(3):
                            nc.tensor.matmul(
                                po[:, :],
                                m_sb[:, cc, ms * 128 : (ms + 1) * 128],
                                xcm[:, cc, tcn * 512 : (tcn + 1) * 512],
                                start=(cc == 0),
                                stop=(cc == 2),
                            )
                        nc.vector.tensor_scalar_add(
                            obuf[:, tcn * 512 : (tcn + 1) * 512],
                            po[:, :],
                            bias_ch[:, ms : ms + 1],
                        )
                        if tcn % 4 == 3:
                            oc = tcn // 4
                            eng = nc.scalar if (ms * NOC + oc) % 2 == 0 else nc.gpsimd
                            eng.dma_start(
                                out=OUT.ap()[ms, :, oc * 2048 : (oc + 1) * 2048],
                                in_=obuf[:, oc * 2048 : (oc + 1) * 2048],
                            )

                if not use_collective:
                    # traffic-parity stand-ins for the AllReduce legs
                    nc.sync.dma_start(out=SQ_OUT.ap(), in_=SQ_IN.ap())
                    nc.sync.dma_start(out=G_OUT.ap(), in_=G_IN.ap())

            if loop_n > 1:
                _eng = mybir.EngineType
                with tc.For_i(
                    0, loop_n, 1, staggered_reset=True,
                    hint_engines=(_eng.PE, _eng.DVE, _eng.Activation, _eng.SP,
                                  _eng.Pool),
                ):
                    body()
            else:
                body()

    nc.compile()
    return nc


_NC_CACHE = {}


def get_nc(loop_n=1, use_collective=True):
    key = (loop_n, use_collective)
    if key not in _NC_CACHE:
        _NC_CACHE[key] = build_nc(loop_n, use_collective)
    return _NC_CACHE[key]


def prep_in_maps(x, W_qkv, temperature_ch, W_out, b_out):
    import ml_dtypes

    x = np.asarray(x, np.float32)
    W_qkv = np.asarray(W_qkv, np.float32)
    W_out = np.asarray(W_out, np.float32)
    b_out = np.asarray(b_out, np.float32)
    temp = np.asarray(temperature_ch, np.float32).reshape(-1)
    x8 = x.astype(ml_dtypes.float8_e4m3)
    xcm_full = np.ascontiguousarray(
        x.transpose(0, 2, 1).astype(ml_dtypes.bfloat16)
    )  # (B, C, N)
    wq = np.ascontiguousarray(W_qkv[0:C].T).reshape(3, 128, C)
    wk = np.ascontiguousarray(W_qkv[C : 2 * C].T).reshape(3, 128, C)
    wqk = np.ascontiguousarray(np.stack([wq, wk]))  # (2, 3, 128, C)
    wv = W_qkv[2 * C : 3 * C].reshape(4, 96, C).transpose(1, 0, 2)  # (96,4,C)
    wo = np.stack(
        [W_out[:, h * DH : (h + 1) * DH].T for h in range(4)]
    ).transpose(1, 0, 2)  # (96, 4, C)
    wvo = np.ascontiguousarray(np.concatenate([wv, wo], axis=1))  # (96,8,C)
    shared = {
        "wqk": wqk, "wvo": wvo,
        "bias": np.ascontiguousarray(b_out.reshape(3, 128)), "temp": temp,
    }
    maps = []
    for core in range(NCORES):
        b, half = core // 2, core % 2
        xs = x8[b, half * NL : (half + 1) * NL]  # (NL, C)
        xt8 = np.ascontiguousarray(
            xs.reshape(NK, 2, 128, C).transpose(0, 2, 1, 3)
        )
        xcm = np.ascontiguousarray(
            xcm_full[b, :, half * NL : (half + 1) * NL]
        ).reshape(3, 128, NL)
        maps.append(dict(shared, xt8=xt8, xcm=xcm))
    return maps


def kernel(**inputs):
    nc = get_nc(1)
    in_maps = prep_in_maps(
        inputs["x"],
        inputs["W_qkv"],
        inputs["temperature_ch"],
        inputs["W_out"],
        inputs["b_out"],
    )
    res = run_bass_kernel_spmd(nc, in_maps, core_ids=list(range(NCORES)))
    out = np.empty((B, NFULL, C), np.float32)
    for core in range(NCORES):
        b, half = core // 2, core % 2
        r = res.results[core]["out"]  # (3, 128, NL) bf16, channel-major
        out[b, half * NL : (half + 1) * NL, :] = (
            r.reshape(C, NL).T.astype(np.float32)
        )
    return out


# revision 32
# speedup vs baseline: 2.7257x; 1.0166x over previous
"""ChannelAttention (LKA3D) Trainium2 Bass kernel, v4.

Problem: B=4, N=16384, C=384, heads=4, head_dim=96.
Reference: qkv = x @ W_qkv.T; per head q,k,v transposed to (d, N);
q,k L2-normalized over N; attn = softmax((q@k.T)*temp, axis=-1);
out = (attn @ v) reassembled to (B,N,C) @ W_out.T + b_out.

Structure: channel attention only needs the d x d gram and per-channel
norms, all bilinear in x. With S = X.T @ X (384x384):
  G_h       = Wq_h.T S Wk_h          (per-head 96x96 logits)
  ||q_c||^2 = diag(Wq.T S Wq),  ||k_d||^2 = diag(Wk.T S Wk)
and the output path folds completely:
  out = V P = X (Wv.T P) = X M,  M = Wv.T (blockdiag(attn).T Wo)

Sharding: core c handles batch c//2, token half c%2 (8192 tokens); a
2-party AllReduce per pair [[0,1],[2,3],[4,5],[6,7]] of the 147KB
stats block (G + norm sums) completes the statistics. The stats chain
runs once per core.

Dtypes (validated vs reference in numpy: rel err 3.7e-3, budget 2e-2):
S from fp8e4 x with DoubleRow matmuls; stats in bf16; out phase bf16 X
/ bf16 M; output downloaded bf16 and upcast on host.

v4 scheduling (from TimelineSim of v3): x loads split in 4 chunks and
weights packed into 2 DMAs so the S build starts ~3us in; activation
function-set loads (1.3us each) pulled off the critical path with dummy
Sqrt/Exp activations; norm rsqrt restructured from [1,768] row ops to
[96,384] wide ops via an outer-product matmul, with temperature folded
into the Exp's per-partition scale; diag products split Pool/DVE;
output stored in 2048-token chunks to shrink the drain tail; loads on
the SP queue, stores on Act/Pool so loop iterations don't block.
"""

import numpy as np
import concourse.bacc as bacc
import concourse.mybir as mybir
from concourse import tile
from concourse.bass_utils import run_bass_kernel_spmd

F32 = mybir.dt.float32
F32R = mybir.dt.float32r
BF16 = mybir.dt.bfloat16
F8 = mybir.dt.float8e4
ALU = mybir.AluOpType
ACTF = mybir.ActivationFunctionType
DR = mybir.MatmulPerfMode.DoubleRow

B = 4
C = 384
NHEADS = 4
DH = 96
NCORES = 8
NFULL = 16384
NL = NFULL // 2        # 8192 tokens per core (one half of one batch)
NK = NL // 256         # 32 fp8 double-row pair tiles
NTC = NL // 512        # 16 out-phase token chunks
NOC = NL // 2048       # 4 output store chunks per ms
STATS_ROWS = 98        # 96 G rows + 2 sq rows


def build_nc(loop_n=1, use_collective=True, unroll=False):
    nc = bacc.Bacc(None, target_bir_lowering=False, debug=False)
    XT8 = nc.dram_tensor("xt8", [NK, 128, 2, C], F8, kind="ExternalInput")
    XCM = nc.dram_tensor("xcm", [3, 128, NL], BF16, kind="ExternalInput")
    WQK = nc.dram_tensor("wqk", [2, 3, 128, C], F32R, kind="ExternalInput")
    WVO = nc.dram_tensor("wvo", [96, 8, C], F32R, kind="ExternalInput")
    BIAS = nc.dram_tensor("bias", [3, 128], F32, kind="ExternalInput")
    TEMP = nc.dram_tensor("temp", [NHEADS], F32R, kind="ExternalInput")
    OUT = nc.dram_tensor("out", [3, 128, NL], BF16, kind="ExternalOutput")
    G_IN = nc.dram_tensor("g_in", [96 * C], F32)
    G_OUT = nc.dram_tensor("g_out", [96 * C], F32)
    SQ_IN = nc.dram_tensor("sq_in", [768], F32)
    SQ_OUT = nc.dram_tensor("sq_out", [768], F32)

    def g_view(t):
        return t.ap().rearrange("(p f) -> p f", p=96)

    with tile.TileContext(nc) as tc:
        with (
            tc.tile_pool(name="wpool", bufs=1) as wpool,
            tc.tile_pool(name="xpool", bufs=1) as xpool,
            tc.tile_pool(name="xcpool", bufs=2) as xcpool,
            tc.tile_pool(name="spool", bufs=1) as spool,
            tc.tile_pool(name="apool", bufs=1) as apool,
            tc.tile_pool(name="gpool", bufs=1) as gpool,
            tc.tile_pool(name="p2pool", bufs=1) as p2pool,
            tc.tile_pool(name="opool", bufs=3) as opool,
            tc.tile_pool(name="pS", bufs=1, space="PSUM") as pS,
            tc.tile_pool(name="pA", bufs=2, space="PSUM") as pA,
            tc.tile_pool(name="pO", bufs=3, space="PSUM") as pO,
        ):
            # ---- weights: loaded once (resident across loop iterations) ----
            wqk_sb = wpool.tile([128, 2, 3, C], F32R, name="wqk", tag="wqk")
            nc.scalar.dma_start(
                out=wqk_sb[:, :, :, :],
                in_=WQK.ap().rearrange("a b p c -> p a b c"),
            )
            wq_sb, wk_sb = wqk_sb[:, 0], wqk_sb[:, 1]
            wvo_sb = wpool.tile([96, 8, C], F32R, name="wvo", tag="wvo")
            nc.scalar.dma_start(out=wvo_sb[:, :, :], in_=WVO.ap())
            bias_ch = wpool.tile([128, 3], F32, name="bias", tag="bias")
            nc.scalar.dma_start(out=bias_ch[:, :], in_=BIAS.ap().rearrange("a p -> p a"))
            temp_sb = wpool.tile([1, NHEADS], F32R, name="temp", tag="temp")
            nc.scalar.dma_start(out=temp_sb[:, :], in_=TEMP.ap()[None, :])
            ones = wpool.tile([128, 1], BF16, name="ones", tag="ones")
            nc.vector.memset(ones[:, :], 1.0)
            ones_row = wpool.tile([1, 96], F32R, name="onesr", tag="onesr")
            nc.vector.memset(ones_row[:, :].bitcast(F32), 1.0)
            wq_bf = wpool.tile([128, 3, C], BF16, name="wqbf", tag="wqbf")
            wk_bf = wpool.tile([128, 3, C], BF16, name="wkbf", tag="wkbf")
            nc.vector.tensor_copy(wq_bf[:, :, :], wq_sb[:, :, :].bitcast(F32))
            nc.vector.tensor_copy(wk_bf[:, :, :], wk_sb[:, :, :].bitcast(F32))
            # temp broadcast to 96 partitions: temp_col = ones_row.T @ temp
            ptc = pA.tile([96, NHEADS], F32, name="ptc", tag="pa")
            nc.tensor.matmul(
                ptc[:, :], ones_row[:, :], temp_sb[:, :],
                start=True, stop=True,
            )
            temp_col = wpool.tile([96, NHEADS], F32, name="tcol", tag="tcol")
            nc.scalar.copy(temp_col[:, :], ptc[:, :])
            # act-table priming: start in the sqrt set
            dummy = wpool.tile([1, 8], F32, name="dummy", tag="dummy")
            nc.vector.memset(dummy[:, :], 1.0)
            dscr = wpool.tile([1, 8], F32, name="dscr", tag="dscr")
            nc.scalar.sqrt(dscr[:, :], dummy[:, :])

            def body():
                # ---- input loads (inside body for loop-timing parity) ------
                xt8 = xpool.tile([128, NK, 2, C], F8, name="xt8", tag="xt8")
                for i in range(4):
                    nc.sync.dma_start(
                        out=xt8[:, i * 8 : (i + 1) * 8, :, :],
                        in_=XT8.ap()[i * 8 : (i + 1) * 8].rearrange(
                            "k p t c -> p k t c"
                        ),
                    )
                # xcm double-buffers across the two traced bodies inside the
                # hardware loop, so this body's load only WARs against the
                # body-before-last's out phase (long done) and transfers land
                # in the bus-idle stats window. 2048-token chunks keep any
                # head-of-line blocking of the small stats DMAs under 1.5us.
                xcm = xcpool.tile([128, 3, NL], BF16, name="xcm", tag="xcm")
                for i in range(4):
                    sl = slice(i * 2048, (i + 1) * 2048)
                    for cc in range(3):
                        nc.sync.dma_start(out=xcm[:, cc, sl], in_=XCM.ap()[cc, :, sl])

                # ---- S = X^T X via fp8 DoubleRow (256 tokens/matmul) -------
                sacc = [
                    pS.tile([128, C], F32, name=f"s{i}", tag=f"ps{i}")
                    for i in range(3)
                ]
                for k in range(NK):
                    for cc in range(3):
                        nc.tensor.matmul(
                            sacc[cc][:, :],
                            xt8[:, k, :, cc * 128 : (cc + 1) * 128],
                            xt8[:, k, :, :],
                            start=(k == 0),
                            stop=(k == NK - 1),
                            perf_mode=DR,
                        )
                s_sb = spool.tile([128, 3, C], F32R, name="s", tag="s")
                nc.scalar.copy(s_sb[:, 0, :], sacc[0][:, :])
                with nc.allow_low_precision(reason="f32r matmul operand"):
                    nc.vector.tensor_copy(s_sb[:, 1, :], sacc[1][:, :])
                nc.scalar.copy(s_sb[:, 2, :], sacc[2][:, :])

                # ---- A = S @ Wk, A' = S @ Wq (bf16 results) ----------------
                ab = {}
                for w_sb, nm in ((wk_sb, "a"), (wq_sb, "ap")):
                    a_bf = apool.tile([128, 3, C], BF16, name=nm, tag=nm)
                    for co in range(3):
                        pa = pA.tile([128, C], F32, name="pa", tag="pa")
                        for e in range(3):
                            nc.tensor.matmul(
                                pa[:, :],
                                s_sb[:, e, co * 128 : (co + 1) * 128],
                                w_sb[:, e, :],
                                start=(e == 0),
                                stop=(e == 2),
                            )
                        if co != 1:
                            nc.scalar.copy(a_bf[:, co, :], pa[:, :])
                        else:
                            nc.vector.tensor_copy(a_bf[:, co, :], pa[:, :])
                    ab[nm] = a_bf
                a_bf, ap_bf = ab["a"], ab["ap"]

                # ---- stats block: G rows 0:96, sq rows 96:98 ---------------
                gacc = gpool.tile([96, C], F32, name="gacc", tag="gacc")
                sqrow = gpool.tile([1, 768], F32, name="sqrow", tag="sqrow")
                for h in range(NHEADS):
                    hs = slice(h * DH, (h + 1) * DH)
                    pg = pA.tile([96, 96], F32, name="pg", tag="pa")
                    for cc in range(3):
                        nc.tensor.matmul(
                            pg[:, :],
                            wq_bf[:, cc, hs],
                            a_bf[:, cc, hs],
                            start=(cc == 0),
                            stop=(cc == 2),
                        )
                    nc.vector.tensor_copy(gacc[:, hs], pg[:, :])
                # diag(W.T S W) via elementwise W*A + ones-matmul
                for j, (w_bf, asb) in enumerate(
                    ((wq_bf, ap_bf), (wk_bf, a_bf))
                ):
                    pd = pA.tile([1, C], F32, name="pd", tag="pa")
                    for cc in range(3):
                        prod = gpool.tile(
                            [128, C], BF16, name=f"prod{j}{cc}", tag=f"prod{j}{cc}"
                        )
                        eng = nc.gpsimd if j == 0 else nc.vector
                        eng.tensor_tensor(
                            prod[:, :], w_bf[:, cc, :], asb[:, cc, :], ALU.mult
                        )
                        nc.tensor.matmul(
                            pd[0:1, :],
                            ones[:, :],
                            prod[:, :],
                            start=(cc == 0),
                            stop=(cc == 2),
                        )
                    nc.vector.tensor_copy(sqrow[0:1, j * C : (j + 1) * C], pd[0:1, :])

                # sq reduced first (tiny): its rsqrt chain + the Exp act-table
                # load hide under the big G reduction that follows
                groups = [[2 * i, 2 * i + 1] for i in range(B)]
                nc.scalar.dma_start(out=SQ_IN.ap()[None, :], in_=sqrow[0:1, :])
                if use_collective:
                    nc.gpsimd.collective_compute(
                        "AllReduce", ALU.add, replica_groups=groups,
                        ins=[SQ_IN.ap().opt()], outs=[SQ_OUT.ap().opt()],
                    )
                nc.scalar.dma_start(out=g_view(G_IN), in_=gacc[:, :])
                if use_collective:
                    nc.gpsimd.collective_compute(
                        "AllReduce", ALU.add, replica_groups=groups,
                        ins=[G_IN.ap().opt()], outs=[G_OUT.ap().opt()],
                    )
                else:
                    # traffic-parity stand-ins for the AllReduce legs; they
                    # also throttle the SP queue so the next body's loads
                    # reach the bus only after this body's stats DMAs
                    nc.sync.dma_start(out=SQ_OUT.ap(), in_=SQ_IN.ap())
                    nc.sync.dma_start(out=G_OUT.ap(), in_=G_IN.ap())
                g_src = G_OUT if use_collective else G_IN
                sq_src = SQ_OUT if use_collective else SQ_IN
                g_all = gpool.tile([96, C], F32, name="gall", tag="gall")
                sq_all = gpool.tile([1, 768], F32, name="sqall", tag="sqall")
                nc.gpsimd.dma_start(out=sq_all[0:1, :], in_=sq_src.ap()[None, :])
                nc.scalar.dma_start(out=g_all[:, :], in_=g_view(g_src))

                # ---- attention weights: logits = G * rsqrt(sqq x sqk) ------
                # psc_sq[c, h*96+d] = sqq[h*96+c] * sqk[h*96+d] via outer mm
                sq_bf = gpool.tile([1, 768], BF16, name="sqbf", tag="sqbf")
                nc.vector.tensor_copy(sq_bf[0:1, :], sq_all[0:1, :])
                psq = pA.tile([96, C], F32, name="psq", tag="pa")
                for h in range(NHEADS):
                    hs = slice(h * DH, (h + 1) * DH)
                    nc.tensor.matmul(
                        psq[:, hs],
                        sq_bf[0:1, hs],
                        sq_bf[0:1, 384 + h * DH : 384 + (h + 1) * DH],
                        start=True,
                        stop=True,
                    )
                vs = p2pool.tile([96, C], F32, name="vs", tag="vs")
                nc.vector.tensor_scalar_max(vs[:, :], psq[:, :], 1e-24)
                nc.scalar.sqrt(vs[:, :], vs[:, :])
                # preload the Exp table while DVE computes recip/logits
                nc.scalar.activation(dscr[:, :], dummy[:, :], ACTF.Exp)
                rsqm = p2pool.tile([96, C], F32, name="rsqm", tag="rsqm")
                nc.vector.reciprocal(rsqm[:, :], vs[:, :])
                logit = p2pool.tile([96, C], F32, name="logit", tag="logit")
                nc.vector.tensor_tensor(
                    logit[:, :], g_all[:, :], rsqm[:, :], ALU.mult
                )
                expt = p2pool.tile([96, C], F32, name="expt", tag="expt")
                den = p2pool.tile([96, NHEADS], F32, name="den", tag="den")
                for h in range(NHEADS):
                    hs = slice(h * DH, (h + 1) * DH)
                    nc.scalar.activation(
                        expt[:, hs], logit[:, hs], ACTF.Exp,
                        scale=temp_col[:, h : h + 1],
                        accum_out=den[:, h : h + 1],
                    )
                denr = p2pool.tile([96, NHEADS], F32, name="denr", tag="denr")
                nc.vector.reciprocal(denr[:, :], den[:, :])
                p_sb = p2pool.tile([96, NHEADS, C], F32R, name="p", tag="p")
                for h in range(NHEADS):
                    hs = slice(h * DH, (h + 1) * DH)
                    attn = p2pool.tile([96, 96], F32R, name="attn", tag="attn")
                    with nc.allow_low_precision(reason="f32r matmul operand"):
                        nc.vector.tensor_scalar_mul(
                            attn[:, :], expt[:, hs], denr[:, h : h + 1]
                        )
                    pp = pA.tile([96, C], F32, name="pp", tag="pa")
                    nc.tensor.matmul(
                        pp[:, :], attn[:, :], wvo_sb[:, 4 + h, :],
                        start=True, stop=True,
                    )
                    nc.scalar.copy(p_sb[:, h, :], pp[:, :])
                m_sb = p2pool.tile([128, 3, C], BF16, name="m", tag="m")
                for co in range(3):
                    pm = pA.tile([128, C], F32, name="pm", tag="pa")
                    for h in range(NHEADS):
                        nc.tensor.matmul(
                            pm[:, :],
                            wvo_sb[:, h, co * 128 : (co + 1) * 128],
                            p_sb[:, h, :],
                            start=(h == 0),
                            stop=(h == NHEADS - 1),
                        )
                    nc.scalar.copy(m_sb[:, co, :], pm[:, :])
                # flip the act table back to the sqrt set during the out phase
                nc.scalar.sqrt(dscr[:, :], dummy[:, :])

                # ---- out.T = M.T X (+bias), bf16, 2048-token store chunks --
                for ms in range(3):
                    for oc in range(NOC):
                        obuf = opool.tile([128, 2048], BF16, name="osb", tag="osb")
                        for tj in range(4):
                            tcn = oc * 4 + tj
                            po = pO.tile([128, 512], F32, name="po", tag="po")
                            for cc in range(3):
                                nc.tensor.matmul(
                                    po[:, :],
                                    m_sb[:, cc, ms * 128 : (ms + 1) * 128],
                                    xcm[:, cc, tcn * 512 : (tcn + 1) * 512],
                                    start=(cc == 0),
                                    stop=(cc == 2),
                                )
                            nc.vector.tensor_scalar_add(
                                obuf[:, tj * 512 : (tj + 1) * 512],
                                po[:, :],
                                bias_ch[:, ms : ms + 1],
                            )
                        eng = nc.scalar if (ms * NOC + oc) % 2 == 0 else nc.gpsimd
                        eng.dma_start(
                            out=OUT.ap()[ms, :, oc * 2048 : (oc + 1) * 2048],
                            in_=obuf[:, :],
                        )

            if loop_n > 1 and unroll:
                for _ in range(loop_n):
                    body()
            elif loop_n > 1:
                # two traced bodies per hardware iteration so the xcm pool
                # ping-pongs (a hardware loop cannot rotate buffers otherwise)
                assert loop_n % 2 == 0, "loop_n must be even"
                _eng = mybir.EngineType
                with tc.For_i(
                    0, loop_n // 2, 1, staggered_reset=True,
                    hint_engines=(_eng.PE, _eng.DVE, _eng.Activation, _eng.SP,
                                  _eng.Pool),
                ):
                    body()
                    body()
            else:
                body()

    nc.compile()
    return nc


_NC_CACHE = {}


def get_nc(loop_n=1, use_collective=True, unroll=False):
    key = (loop_n, use_collective, unroll)
    if key not in _NC_CACHE:
        _NC_CACHE[key] = build_nc(loop_n, use_collective, unroll)
    return _NC_CACHE[key]


def prep_in_maps(x, W_qkv, temperature_ch, W_out, b_out):
    import ml_dtypes

    x = np.asarray(x, np.float32)
    W_qkv = np.asarray(W_qkv, np.float32)
    W_out = np.asarray(W_out, np.float32)
    b_out = np.asarray(b_out, np.float32)
    temp = np.asarray(temperature_ch, np.float32).reshape(-1)
    x8 = x.astype(ml_dtypes.float8_e4m3)
    xcm_full = np.ascontiguousarray(
        x.transpose(0, 2, 1).astype(ml_dtypes.bfloat16)
    )  # (B, C, N)
    wq = np.ascontiguousarray(W_qkv[0:C].T).reshape(3, 128, C)
    wk = np.ascontiguousarray(W_qkv[C : 2 * C].T).reshape(3, 128, C)
    wqk = np.ascontiguousarray(np.stack([wq, wk]))  # (2, 3, 128, C)
    wv = W_qkv[2 * C : 3 * C].reshape(4, 96, C).transpose(1, 0, 2)  # (96,4,C)
    wo = np.stack(
        [W_out[:, h * DH : (h + 1) * DH].T for h in range(4)]
    ).transpose(1, 0, 2)  # (96, 4, C)
    wvo = np.ascontiguousarray(np.concatenate([wv, wo], axis=1))  # (96,8,C)
    shared = {
        "wqk": wqk, "wvo": wvo,
        "bias": np.ascontiguousarray(b_out.reshape(3, 128)), "temp": temp,
    }
    maps = []
    for core in range(NCORES):
        b, half = core // 2, core % 2
        xs = x8[b, half * NL : (half + 1) * NL]  # (NL, C)
        xt8 = np.ascontiguousarray(
            xs.reshape(NK, 2, 128, C).transpose(0, 2, 1, 3)
        )
        xcm = np.ascontiguousarray(
            xcm_full[b, :, half * NL : (half + 1) * NL]
        ).reshape(3, 128, NL)
        maps.append(dict(shared, xt8=xt8, xcm=xcm))
    return maps


def kernel(**inputs):
    nc = get_nc(1)
    in_maps = prep_in_maps(
        inputs["x"],
        inputs["W_qkv"],
        inputs["temperature_ch"],
        inputs["W_out"],
        inputs["b_out"],
    )
    res = run_bass_kernel_spmd(nc, in_maps, core_ids=list(range(NCORES)))
    out = np.empty((B, NFULL, C), np.float32)
    for core in range(NCORES):
        b, half = core // 2, core % 2
        r = res.results[core]["out"]  # (3, 128, NL) bf16, channel-major
        out[b, half * NL : (half + 1) * NL, :] = (
            r.reshape(C, NL).T.astype(np.float32)
        )
    return out
